# revision 58
# baseline (speedup 1.0000x reference)
"""Trainium2 Bass kernel for nn_AttentionBlock (B=8, C=128, H=W=64, A=16).

Data-parallel over batch across 8 NeuronCores (one batch each).
Per core, attention over N=4096 pixels with A=16 attention channels.

Design (199us bf16 baseline -> ~140us):
  - P = exp(S-4) is stored as fp8e5m2 so the PV accumulation AND the
    softmax denominator (Z = ones^T P) run as fp8 DoubleRow matmuls:
    each contracts a PAIR of 128-key tiles (virtual K=256) per 512-col
    stream, halving TensorE streaming for PV+Z (2/3 of baseline PE
    work).  Pairs are addressed with 3D APs [128, 2, m] over a
    16-slot PT ring in SBUF - no interleaved writes anywhere.
  - The exp itself (16.7M elems/core at 1 elem/cycle/lane) would be a
    ~140us ScalarE bottleneck, so it is split across TWO engines:
      * ScalarE: native exp activation -> fp8e5m2, bias=-4
      * VectorE: one fused tensor_scalar (mult,add) producing the
        Schraudolph bit pattern round(5.77*S + 36.68) as uint8 (the
        fp32->uint8 convert saturates [0,255] with RNE - HW verified),
        bitcast to e5m2 ~= exp(S-4) with ~5% weighted-rms error.
        The numerator and denominator use the SAME quantized P, so the
        softmax stays an exact weighted average and errors largely
        cancel; measured end-to-end rel err ~2.8e-3 (gate 2e-2).
    Ops are greedily load-balanced; DVE also owns recip + normalize,
    GPSIMD does the residual add, the out DMA ships bf16.
  - S matmuls (K=16) run 4-per-group concurrently in PE row groups
    0/32/64/96; S PSUM = two [128,1024] tiles per group from a bufs=3
    pool (6 banks) + O + Z = all 8 PSUM banks.  DR flushes are batched
    2 pairs per group with a 1-pair keep-back (LDWEIGHTS only fails
    to hide across row-tiled<->full-array transitions, ~245ns/group
    irreducible).  Pair-exps alternate STRICTLY (S-half A -> ScalarE,
    B -> VectorE): bounding each half's exp latency beats greedy load
    balancing because the S-PSUM WAR chain S(g)->exp(g)->S(g+1.5) is
    the steady-state critical path.
  - x ships as bf16 only (residual from bf16, +1e-3 err), v is
    projected straight into the fp8e4m3 pair layout, q/k stay bf16.
"""

import os
import numpy as np

import concourse.bass as bass
import concourse.mybir as mybir
import concourse.tile as tile
from concourse import bacc
from concourse.bass_utils import run_bass_kernel_spmd

import ml_dtypes

_BF16 = np.dtype(ml_dtypes.bfloat16)
_F8 = np.dtype(ml_dtypes.float8_e4m3)  # v stays e4m3

N_CORES = 8
C = 128
A = 16
B = 8
IC = 512          # query-chunk width (one PSUM bank)
JQ = 4            # key-tiles per S-step (PE row groups)

# Schraudolph fp8e4m3 exp: bits = round(SCH_A*S + SCH_B) (uint8-saturating
# convert), bitcast e4m3 ~= exp(S-4).  SCH_B fitted on N(0,1.28) scores.
SCH_A = 4.0 / float(np.log(2.0))
SCH_B = 36.6769

# per-op engine cost models (ns) used only for static load balancing
def _sa_cost(fd, psum=True):
    return (fd + (172 if psum else 224)) / 1.2


def _dve_cost(fd, psum=True):
    return (fd + (120 if psum else 58)) / 0.96


def build_nc(n=4096):
    f32 = mybir.dt.float32
    bf16 = mybir.dt.bfloat16
    fp8 = mybir.dt.float8e4
    fp8e5 = mybir.dt.float8e5
    u8 = mybir.dt.uint8
    Ident = mybir.ActivationFunctionType.Identity
    Exp = mybir.ActivationFunctionType.Exp
    DR = mybir.MatmulPerfMode.DoubleRow
    MUL = mybir.AluOpType.mult
    ADD = mybir.AluOpType.add

    nj = n // 128        # 32 key tiles
    ni = n // IC         # 8 query chunks
    nx = n // 512        # x chunks (dma/projection granularity)
    npair = nj // 2      # 16 key-tile pairs per chunk

    # pipeline steps: (ic, j0, qlen)
    steps = []
    for ic in range(ni):
        j0 = 0
        while j0 < nj:
            qlen = min(JQ, nj - j0)
            steps.append((ic, j0, qlen))
            j0 += qlen

    nc = bacc.Bacc("TRN2", target_bir_lowering=False, debug=False,
                   num_devices=N_CORES)

    xbf_ext = nc.dram_tensor("x_bf", [C, n], bf16, kind="ExternalInput").ap()
    wq4_ext = nc.dram_tensor("wq4", [C, C], bf16, kind="ExternalInput").ap()
    wk4_ext = nc.dram_tensor("wk4", [C, C], bf16, kind="ExternalInput").ap()
    wvT_ext = nc.dram_tensor("wvT", [C, C], bf16, kind="ExternalInput").ap()
    bq4_ext = nc.dram_tensor("bq4", [C, 1], f32, kind="ExternalInput").ap()
    bk4_ext = nc.dram_tensor("bk4", [C, 1], f32, kind="ExternalInput").ap()
    bv_ext = nc.dram_tensor("bv", [C, 1], f32, kind="ExternalInput").ap()
    out_ext = nc.dram_tensor("out", [C, n], bf16, kind="ExternalOutput").ap()

    def pair3(ap2d):
        return ap2d.rearrange("k (two m) -> k two m", two=2)

    with tile.TileContext(nc) as tc:
        with tc.tile_pool(name="persist", bufs=1) as persist:
            wq4 = persist.tile([C, C], bf16, tag="wq4")
            nc.sync.dma_start(wq4[:], wq4_ext[:])
            wk4 = persist.tile([C, C], bf16, tag="wk4")
            nc.sync.dma_start(wk4[:], wk4_ext[:])
            wvT = persist.tile([C, C], bf16, tag="wvT")
            nc.sync.dma_start(wvT[:], wvT_ext[:])
            bq4_sb = persist.tile([C, 1], f32, tag="bq4_sb")
            nc.sync.dma_start(bq4_sb[:], bq4_ext[:])
            bk4_sb = persist.tile([C, 1], f32, tag="bk4_sb")
            nc.sync.dma_start(bk4_sb[:], bk4_ext[:])
            bv_sb = persist.tile([C, 1], f32, tag="bv_sb")
            nc.sync.dma_start(bv_sb[:], bv_ext[:])

            xf_bf = persist.tile([C, n], bf16, tag="xf_bf")
            # all x blocks on ONE queue: transfers serialize, so the
            # first block (which gates the projection chain) gets full
            # HBM bandwidth instead of 1/4 of it
            for h in range(nx // 2):
                sl = slice(h * 1024, (h + 1) * 1024)
                nc.gpsimd.dma_start(xf_bf[:, sl], xbf_ext[:, sl])

            ones_dr = persist.tile([C, 256], fp8, tag="ones_dr")
            nc.vector.memset(ones_dr[:], 1.0)
            negc = persist.tile([C, 1], f32, tag="negc")
            nc.vector.memset(negc[:], -4.0)

            xr = persist.tile([C, n], f32, tag="xr")
            q4 = persist.tile([C, n], bf16, tag="q4")
            k4 = persist.tile([C, n], bf16, tag="k4")
            # v in fp8, pair-blocked: pair p -> cols [256p:256p+256],
            # key tile 2p in the first 128, tile 2p+1 in the second.
            vT = persist.tile([C, n], fp8, tag="vT")

            # PT ring: 16 slots of 512 queries (8 pairs in flight);
            # pair p lives at cols [1024*(p%8) : +1024].
            pt_ring = persist.tile([C, 16 * 512], fp8e5, tag="pt_ring")

            # --- projection phase (pipelined per 512-column chunk) ---
            with tc.tile_pool(name="proj_ps", bufs=3, space="PSUM") as pps:
                for h in range(nx):
                    sl = slice(h * 512, (h + 1) * 512)
                    qp = pps.tile([C, 512], f32, tag="qkp", name=f"qp_{h}")
                    nc.tensor.matmul(qp[:], wq4[:], xf_bf[:, sl],
                                     start=True, stop=True)
                    nc.scalar.activation(q4[:, sl], qp[:], Ident,
                                         bias=bq4_sb[:])
                    kp = pps.tile([C, 512], f32, tag="qkp", name=f"kp_{h}")
                    nc.tensor.matmul(kp[:], wk4[:], xf_bf[:, sl],
                                     start=True, stop=True)
                    nc.vector.tensor_scalar_add(k4[:, sl], kp[:], bk4_sb[:])
                    for pp in (2 * h, 2 * h + 1):
                        vp = pps.tile([C, 256], f32, tag="vp",
                                      name=f"vp_{pp}")
                        for half in range(2):
                            jt = 2 * pp + half
                            vsl = slice(jt * 128, (jt + 1) * 128)
                            nc.tensor.matmul(vp[:, half * 128:half * 128 + 128],
                                             xf_bf[:, vsl], wvT[:],
                                             start=True, stop=True)
                        dst = vT[:, 256 * pp:256 * pp + 256]
                        if pp % 2 == 0:
                            nc.scalar.activation(dst, vp[:], Ident)
                        else:
                            nc.vector.tensor_copy(dst, vp[:])

            # --- main attention loop ---
            with tc.tile_pool(name="ep_pool", bufs=2) as epp, \
                 tc.tile_pool(name="ps_S", bufs=3, space="PSUM") as psS, \
                 tc.tile_pool(name="ps_O", bufs=1, space="PSUM") as psO, \
                 tc.tile_pool(name="ps_Z", bufs=1, space="PSUM") as psZ:

                    OZ = {}
                    sa_t = [0.0]   # engine load accumulators (ns)
                    dve_t = [0.0]

                    def assign(sa_cost, dve_cost):
                        """greedy: pick engine finishing first; update load."""
                        if sa_t[0] + sa_cost <= dve_t[0] + dve_cost:
                            sa_t[0] += sa_cost
                            return "sa"
                        dve_t[0] += dve_cost
                        return "dve"

                    def emit_exp(ic, t0, cnt, src_ap, eng=None):
                        """exp of `cnt` key tiles starting at global tile t0
                        of chunk ic, from PSUM slice -> PT ring slots."""
                        s0 = t0 % 16
                        dst = pt_ring[:, 512 * s0:512 * (s0 + cnt)]
                        fd = 512 * cnt
                        if eng is None:
                            eng = assign(_sa_cost(fd), _dve_cost(fd))
                        if eng == "sa":
                            nc.scalar.activation(dst, src_ap, Exp,
                                                 bias=negc[:])
                        else:
                            nc.vector.tensor_scalar(dst.bitcast(u8), src_ap,
                                                    SCH_A, SCH_B, MUL, ADD)

                    def flush_pair(ic, p):
                        O_ps, Z_ps = OZ[ic]
                        PT = pt_ring[:, 1024 * (p % 8):1024 * (p % 8) + 1024]
                        first = p == 0
                        last = p == npair - 1
                        vsl = vT[:, 256 * p:256 * p + 256]
                        nc.tensor.matmul(O_ps[:], pair3(vsl), pair3(PT),
                                         start=first, stop=last,
                                         perf_mode=DR)
                        nc.tensor.matmul(Z_ps[:], pair3(ones_dr[:]),
                                         pair3(PT),
                                         start=first, stop=last,
                                         perf_mode=DR)
                        if last:
                            isl = slice(ic * IC, (ic + 1) * IC)
                            recip = epp.tile([C, IC], f32, tag="recip")
                            nc.vector.reciprocal_approx_fast(recip[:],
                                                             Z_ps[:])
                            o1 = epp.tile([C, IC], f32, tag="o1")
                            nc.vector.tensor_mul(o1[:], O_ps[:], recip[:])
                            o2 = epp.tile([C, IC], bf16, tag="o2")
                            if ic == ni - 1:
                                nc.vector.tensor_add(o2[:], o1[:], xr[:, isl])
                            else:
                                nc.gpsimd.tensor_add(o2[:], o1[:], xr[:, isl])
                            nc.sync.dma_start(out_ext[:, isl], o2[:])
                            dve_t[0] += 1320.0

                    # residual base xr = x + bv, one 512-chunk at a time,
                    # greedily placed (needed only at each chunk's end).
                    def emit_xr(h):
                        sl = slice(h * 512, (h + 1) * 512)
                        dve_t[0] += 330.0
                        nc.vector.tensor_scalar_add(xr[:, sl], xf_bf[:, sl],
                                                    bv_sb[:])

                    flushed = {}
                    pend_exp = None    # (ic, list of (t0, cnt), S_ps)
                    for (ic, j0, qlen) in steps:
                        if j0 == 0:
                            O_ps = psO.tile([C, IC], f32, tag="O_ps")
                            Z_ps = psZ.tile([C, IC], f32, tag="Z_ps")
                            OZ[ic] = (O_ps, Z_ps)
                            flushed[ic] = 0
                            emit_xr(ic)
                        isl = slice(ic * IC, (ic + 1) * IC)
                        # 4 key tiles per step in the 4 PE row groups, S
                        # landing in two [128, 1024] PSUM tiles (2 banks ea).
                        sAB = []
                        for half in range(2):
                            S_ps = psS.tile([128, 2 * IC], f32, tag="S_ps",
                                            name=f"S_{ic}_{j0}_{half}")
                            sAB.append(S_ps)
                            for r in (2 * half, 2 * half + 1):
                                jt = j0 + r
                                p0 = 32 * r
                                nc.tensor.matmul(
                                    S_ps[:, (r % 2) * IC:(r % 2 + 1) * IC],
                                    k4[p0:p0 + A, jt * 128:(jt + 1) * 128],
                                    q4[p0:p0 + A, isl],
                                    start=True, stop=True,
                                    tile_position=(p0, 0))
                        # flush pairs completed by the previous step's exps
                        if pend_exp is not None:
                            pic, plast = pend_exp
                            pdone = (plast + 1) // 2
                            keep = 0 if pdone >= npair else 1
                            while flushed[pic] < pdone - keep:
                                flush_pair(pic, flushed[pic])
                                flushed[pic] += 1
                        # two pair-exp ops per step, strictly one per
                        # engine so each S-half's exp latency is bounded
                        for half in range(2):
                            emit_exp(ic, j0 + 2 * half, 2, sAB[half][:],
                                     eng="sa" if half == 0 else "dve")
                        pend_exp = (ic, j0 + qlen - 1)

                    pic, plast = pend_exp
                    while flushed[pic] < npair:
                        flush_pair(pic, flushed[pic])
                        flushed[pic] += 1

    nc.compile()
    return nc


_NC_CACHE = {}


def _get_nc(n=4096):
    if n not in _NC_CACHE:
        _NC_CACHE[n] = build_nc(n)
    return _NC_CACHE[n]


def _spread(w):
    """[A, C] weight -> [C, C] lhsT with W.T in 4 row-group column bands."""
    out = np.zeros((C, C), dtype=np.float32)
    for r in range(4):
        out[:, 32 * r:32 * r + A] = w.T
    return out.astype(_BF16)


def _spread_bias(b):
    out = np.zeros((C, 1), dtype=np.float32)
    for r in range(4):
        out[32 * r:32 * r + A, 0] = b
    return out


def kernel(x, Wq, bq, Wk, bk, Wv, bv):
    x = np.asarray(x, dtype=np.float32)
    Wq = np.asarray(Wq, dtype=np.float32)
    bq = np.asarray(bq, dtype=np.float32)
    Wk = np.asarray(Wk, dtype=np.float32)
    bk = np.asarray(bk, dtype=np.float32)
    Wv = np.asarray(Wv, dtype=np.float32)
    bv = np.asarray(bv, dtype=np.float32)

    b, c, hh, ww = x.shape
    n = hh * ww
    assert (b, c) == (B, C) and n == 4096

    nc = _get_nc(n)

    in_common = {
        "wq4": _spread(Wq),
        "wk4": _spread(Wk),
        "wvT": np.ascontiguousarray(Wv.T).astype(_BF16),
        "bq4": _spread_bias(bq),
        "bk4": _spread_bias(bk),
        "bv": np.ascontiguousarray(bv.reshape(C, 1)),
    }
    in_maps = []
    for i in range(B):
        xi = np.ascontiguousarray(x[i].reshape(C, n))
        in_maps.append({"x_bf": xi.astype(_BF16), **in_common})

    trace = bool(int(os.environ.get("BASS_KERNEL_PROFILE", "0")))
    res = run_bass_kernel_spmd(nc, in_maps, core_ids=list(range(N_CORES)),
                               trace=trace)
    if trace:
        kernel.last_exec_time_ns = res.exec_time_ns
        kernel.last_results = res

    out = np.stack([res.results[i]["out"].astype(np.float32).reshape(C, hh, ww)
                    for i in range(B)])
    return out


# revision 59
# speedup vs baseline: 1.0408x; 1.0408x over previous
"""Trainium2 Bass kernel for nn_AttentionBlock (B=8, C=128, H=W=64, A=16).

Data-parallel over batch across 8 NeuronCores (one batch each).
Per core, attention over N=4096 pixels with A=16 attention channels.

Design (199us bf16 baseline -> ~140us):
  - P = exp(S-4) is stored as fp8e5m2 so the PV accumulation AND the
    softmax denominator (Z = ones^T P) run as fp8 DoubleRow matmuls:
    each contracts a PAIR of 128-key tiles (virtual K=256) per 512-col
    stream, halving TensorE streaming for PV+Z (2/3 of baseline PE
    work).  Pairs are addressed with 3D APs [128, 2, m] over a
    16-slot PT ring in SBUF - no interleaved writes anywhere.
  - The exp itself (16.7M elems/core at 1 elem/cycle/lane) would be a
    ~140us ScalarE bottleneck, so it is split across TWO engines:
      * ScalarE: native exp activation -> fp8e5m2, bias=-4
      * VectorE: one fused tensor_scalar (mult,add) producing the
        Schraudolph bit pattern round(5.77*S + 36.68) as uint8 (the
        fp32->uint8 convert saturates [0,255] with RNE - HW verified),
        bitcast to e5m2 ~= exp(S-4) with ~5% weighted-rms error.
        The numerator and denominator use the SAME quantized P, so the
        softmax stays an exact weighted average and errors largely
        cancel; measured end-to-end rel err ~2.8e-3 (gate 2e-2).
    Ops are greedily load-balanced; DVE also owns recip + normalize,
    GPSIMD does the residual add, the out DMA ships bf16.
  - S matmuls (K=16) run 4-per-group concurrently in PE row groups
    0/32/64/96; S PSUM = two [128,1024] tiles per group from a bufs=3
    pool (6 banks) + O + Z = all 8 PSUM banks.  DR flushes are batched
    2 pairs per group with a 1-pair keep-back (LDWEIGHTS only fails
    to hide across row-tiled<->full-array transitions, ~245ns/group
    irreducible).  Pair-exps alternate STRICTLY (S-half A -> ScalarE,
    B -> VectorE): bounding each half's exp latency beats greedy load
    balancing because the S-PSUM WAR chain S(g)->exp(g)->S(g+1.5) is
    the steady-state critical path.
  - x ships as bf16 only (residual from bf16, +1e-3 err), v is
    projected straight into the fp8e4m3 pair layout, q/k stay bf16.
"""

import os
import numpy as np

import concourse.bass as bass
import concourse.mybir as mybir
import concourse.tile as tile
from concourse import bacc
from concourse.bass_utils import run_bass_kernel_spmd

import ml_dtypes

_BF16 = np.dtype(ml_dtypes.bfloat16)
_F8 = np.dtype(ml_dtypes.float8_e4m3)  # v stays e4m3

N_CORES = 8
C = 128
A = 16
B = 8
IC = 512          # query-chunk width (one PSUM bank)
JQ = 4            # key-tiles per S-step (PE row groups)

# Schraudolph fp8e4m3 exp: bits = round(SCH_A*S + SCH_B) (uint8-saturating
# convert), bitcast e4m3 ~= exp(S-4).  SCH_B fitted on N(0,1.28) scores.
SCH_A = 4.0 / float(np.log(2.0))
SCH_B = 36.6769

# per-op engine cost models (ns) used only for static load balancing
def _sa_cost(fd, psum=True):
    return (fd + (172 if psum else 224)) / 1.2


def _dve_cost(fd, psum=True):
    return (fd + (120 if psum else 58)) / 0.96


def build_nc(n=4096):
    f32 = mybir.dt.float32
    bf16 = mybir.dt.bfloat16
    fp8 = mybir.dt.float8e4
    fp8e5 = mybir.dt.float8e5
    u8 = mybir.dt.uint8
    Ident = mybir.ActivationFunctionType.Identity
    Exp = mybir.ActivationFunctionType.Exp
    DR = mybir.MatmulPerfMode.DoubleRow
    MUL = mybir.AluOpType.mult
    ADD = mybir.AluOpType.add

    nj = n // 128        # 32 key tiles
    ni = n // IC         # 8 query chunks
    nx = n // 512        # x chunks (dma/projection granularity)
    npair = nj // 2      # 16 key-tile pairs per chunk

    # pipeline steps: (ic, j0, qlen)
    steps = []
    for ic in range(ni):
        j0 = 0
        while j0 < nj:
            qlen = min(JQ, nj - j0)
            steps.append((ic, j0, qlen))
            j0 += qlen

    nc = bacc.Bacc("TRN2", target_bir_lowering=False, debug=False,
                   num_devices=N_CORES)

    xbf_ext = nc.dram_tensor("x_bf", [C, n], bf16, kind="ExternalInput").ap()
    wq4_ext = nc.dram_tensor("wq4", [C, C], bf16, kind="ExternalInput").ap()
    wk4_ext = nc.dram_tensor("wk4", [C, C], bf16, kind="ExternalInput").ap()
    wvT_ext = nc.dram_tensor("wvT", [C, C], bf16, kind="ExternalInput").ap()
    bq4_ext = nc.dram_tensor("bq4", [C, 1], f32, kind="ExternalInput").ap()
    bk4_ext = nc.dram_tensor("bk4", [C, 1], f32, kind="ExternalInput").ap()
    bv_ext = nc.dram_tensor("bv", [C, 1], f32, kind="ExternalInput").ap()
    out_ext = nc.dram_tensor("out", [C, n], bf16, kind="ExternalOutput").ap()

    def pair3(ap2d):
        return ap2d.rearrange("k (two m) -> k two m", two=2)

    with tile.TileContext(nc) as tc:
        with tc.tile_pool(name="persist", bufs=1) as persist:
            wq4 = persist.tile([C, C], bf16, tag="wq4")
            nc.sync.dma_start(wq4[:], wq4_ext[:])
            wk4 = persist.tile([C, C], bf16, tag="wk4")
            nc.sync.dma_start(wk4[:], wk4_ext[:])
            wvT = persist.tile([C, C], bf16, tag="wvT")
            nc.sync.dma_start(wvT[:], wvT_ext[:])
            bq4_sb = persist.tile([C, 1], f32, tag="bq4_sb")
            nc.sync.dma_start(bq4_sb[:], bq4_ext[:])
            bk4_sb = persist.tile([C, 1], f32, tag="bk4_sb")
            nc.sync.dma_start(bk4_sb[:], bk4_ext[:])
            bv_sb = persist.tile([C, 1], f32, tag="bv_sb")
            nc.sync.dma_start(bv_sb[:], bv_ext[:])

            xf_bf = persist.tile([C, n], bf16, tag="xf_bf")
            dma_engines = [nc.gpsimd, nc.sync]
            for h in range(nx // 2):
                sl = slice(h * 1024, (h + 1) * 1024)
                dma_engines[h % 2].dma_start(xf_bf[:, sl], xbf_ext[:, sl])

            ones_dr = persist.tile([C, 256], fp8, tag="ones_dr")
            nc.vector.memset(ones_dr[:], 1.0)
            negc = persist.tile([C, 1], f32, tag="negc")
            nc.vector.memset(negc[:], -4.0)

            xr = persist.tile([C, n], f32, tag="xr")
            q4 = persist.tile([C, n], bf16, tag="q4")
            k4 = persist.tile([C, n], bf16, tag="k4")
            # v in fp8, pair-blocked: pair p -> cols [256p:256p+256],
            # key tile 2p in the first 128, tile 2p+1 in the second.
            vT = persist.tile([C, n], fp8, tag="vT")

            # PT ring: 16 slots of 512 queries (8 pairs in flight);
            # pair p lives at cols [1024*(p%8) : +1024].
            pt_ring = persist.tile([C, 16 * 512], fp8e5, tag="pt_ring")

            # --- projection phase (pipelined per 512-column chunk) ---
            with tc.tile_pool(name="proj_ps", bufs=3, space="PSUM") as pps:
                for h in range(nx):
                    sl = slice(h * 512, (h + 1) * 512)
                    qp = pps.tile([C, 512], f32, tag="qkp", name=f"qp_{h}")
                    nc.tensor.matmul(qp[:], wq4[:], xf_bf[:, sl],
                                     start=True, stop=True)
                    nc.scalar.activation(q4[:, sl], qp[:], Ident,
                                         bias=bq4_sb[:])
                    kp = pps.tile([C, 512], f32, tag="qkp", name=f"kp_{h}")
                    nc.tensor.matmul(kp[:], wk4[:], xf_bf[:, sl],
                                     start=True, stop=True)
                    nc.vector.tensor_scalar_add(k4[:, sl], kp[:], bk4_sb[:])
                    for pp in (2 * h, 2 * h + 1):
                        vp = pps.tile([C, 256], f32, tag="vp",
                                      name=f"vp_{pp}")
                        for half in range(2):
                            jt = 2 * pp + half
                            vsl = slice(jt * 128, (jt + 1) * 128)
                            nc.tensor.matmul(vp[:, half * 128:half * 128 + 128],
                                             xf_bf[:, vsl], wvT[:],
                                             start=True, stop=True)
                        dst = vT[:, 256 * pp:256 * pp + 256]
                        if pp % 2 == 0:
                            nc.scalar.activation(dst, vp[:], Ident)
                        else:
                            nc.vector.tensor_copy(dst, vp[:])

            # --- main attention loop ---
            with tc.tile_pool(name="ep_pool", bufs=2) as epp, \
                 tc.tile_pool(name="ps_S", bufs=3, space="PSUM") as psS, \
                 tc.tile_pool(name="ps_O", bufs=1, space="PSUM") as psO, \
                 tc.tile_pool(name="ps_Z", bufs=1, space="PSUM") as psZ:

                    OZ = {}
                    sa_t = [0.0]   # engine load accumulators (ns)
                    dve_t = [0.0]

                    def assign(sa_cost, dve_cost):
                        """greedy: pick engine finishing first; update load."""
                        if sa_t[0] + sa_cost <= dve_t[0] + dve_cost:
                            sa_t[0] += sa_cost
                            return "sa"
                        dve_t[0] += dve_cost
                        return "dve"

                    def emit_exp(ic, t0, cnt, src_ap, eng=None):
                        """exp of `cnt` key tiles starting at global tile t0
                        of chunk ic, from PSUM slice -> PT ring slots."""
                        s0 = t0 % 16
                        dst = pt_ring[:, 512 * s0:512 * (s0 + cnt)]
                        fd = 512 * cnt
                        if eng is None:
                            eng = assign(_sa_cost(fd), _dve_cost(fd))
                        if eng == "sa":
                            nc.scalar.activation(dst, src_ap, Exp,
                                                 bias=negc[:])
                        else:
                            nc.vector.tensor_scalar(dst.bitcast(u8), src_ap,
                                                    SCH_A, SCH_B, MUL, ADD)

                    def flush_pair(ic, p):
                        O_ps, Z_ps = OZ[ic]
                        PT = pt_ring[:, 1024 * (p % 8):1024 * (p % 8) + 1024]
                        first = p == 0
                        last = p == npair - 1
                        vsl = vT[:, 256 * p:256 * p + 256]
                        nc.tensor.matmul(O_ps[:], pair3(vsl), pair3(PT),
                                         start=first, stop=last,
                                         perf_mode=DR)
                        nc.tensor.matmul(Z_ps[:], pair3(ones_dr[:]),
                                         pair3(PT),
                                         start=first, stop=last,
                                         perf_mode=DR)
                        if last:
                            isl = slice(ic * IC, (ic + 1) * IC)
                            recip = epp.tile([C, IC], f32, tag="recip")
                            nc.vector.reciprocal_approx_fast(recip[:],
                                                             Z_ps[:])
                            o1 = epp.tile([C, IC], f32, tag="o1")
                            nc.vector.tensor_mul(o1[:], O_ps[:], recip[:])
                            o2 = epp.tile([C, IC], bf16, tag="o2")
                            if ic == ni - 1:
                                nc.vector.tensor_add(o2[:], o1[:], xr[:, isl])
                            else:
                                nc.gpsimd.tensor_add(o2[:], o1[:], xr[:, isl])
                            nc.sync.dma_start(out_ext[:, isl], o2[:])
                            dve_t[0] += 1320.0

                    # residual base xr = x + bv, one 512-chunk at a time,
                    # greedily placed (needed only at each chunk's end).
                    def emit_xr(h):
                        sl = slice(h * 512, (h + 1) * 512)
                        dve_t[0] += 330.0
                        nc.vector.tensor_scalar_add(xr[:, sl], xf_bf[:, sl],
                                                    bv_sb[:])

                    flushed = {}
                    pend_exp = None    # (ic, list of (t0, cnt), S_ps)
                    for (ic, j0, qlen) in steps:
                        if j0 == 0:
                            O_ps = psO.tile([C, IC], f32, tag="O_ps")
                            Z_ps = psZ.tile([C, IC], f32, tag="Z_ps")
                            OZ[ic] = (O_ps, Z_ps)
                            flushed[ic] = 0
                            emit_xr(ic)
                        isl = slice(ic * IC, (ic + 1) * IC)
                        # 4 key tiles per step in the 4 PE row groups, S
                        # landing in two [128, 1024] PSUM tiles (2 banks ea).
                        sAB = []
                        for half in range(2):
                            S_ps = psS.tile([128, 2 * IC], f32, tag="S_ps",
                                            name=f"S_{ic}_{j0}_{half}")
                            sAB.append(S_ps)
                            for r in (2 * half, 2 * half + 1):
                                jt = j0 + r
                                p0 = 32 * r
                                nc.tensor.matmul(
                                    S_ps[:, (r % 2) * IC:(r % 2 + 1) * IC],
                                    k4[p0:p0 + A, jt * 128:(jt + 1) * 128],
                                    q4[p0:p0 + A, isl],
                                    start=True, stop=True,
                                    tile_position=(p0, 0))
                        # flush pairs completed by the previous step's exps
                        if pend_exp is not None:
                            pic, plast = pend_exp
                            pdone = (plast + 1) // 2
                            keep = 0 if pdone >= npair else 1
                            while flushed[pic] < pdone - keep:
                                flush_pair(pic, flushed[pic])
                                flushed[pic] += 1
                        # two pair-exp ops per step, strictly one per
                        # engine so each S-half's exp latency is bounded
                        for half in range(2):
                            emit_exp(ic, j0 + 2 * half, 2, sAB[half][:],
                                     eng="sa" if half == 0 else "dve")
                        pend_exp = (ic, j0 + qlen - 1)

                    pic, plast = pend_exp
                    while flushed[pic] < npair:
                        flush_pair(pic, flushed[pic])
                        flushed[pic] += 1

    nc.compile()
    return nc


_NC_CACHE = {}


def _get_nc(n=4096):
    if n not in _NC_CACHE:
        _NC_CACHE[n] = build_nc(n)
    return _NC_CACHE[n]


def _spread(w):
    """[A, C] weight -> [C, C] lhsT with W.T in 4 row-group column bands."""
    out = np.zeros((C, C), dtype=np.float32)
    for r in range(4):
        out[:, 32 * r:32 * r + A] = w.T
    return out.astype(_BF16)


def _spread_bias(b):
    out = np.zeros((C, 1), dtype=np.float32)
    for r in range(4):
        out[32 * r:32 * r + A, 0] = b
    return out


def kernel(x, Wq, bq, Wk, bk, Wv, bv):
    x = np.asarray(x, dtype=np.float32)
    Wq = np.asarray(Wq, dtype=np.float32)
    bq = np.asarray(bq, dtype=np.float32)
    Wk = np.asarray(Wk, dtype=np.float32)
    bk = np.asarray(bk, dtype=np.float32)
    Wv = np.asarray(Wv, dtype=np.float32)
    bv = np.asarray(bv, dtype=np.float32)

    b, c, hh, ww = x.shape
    n = hh * ww
    assert (b, c) == (B, C) and n == 4096

    nc = _get_nc(n)

    in_common = {
        "wq4": _spread(Wq),
        "wk4": _spread(Wk),
        "wvT": np.ascontiguousarray(Wv.T).astype(_BF16),
        "bq4": _spread_bias(bq),
        "bk4": _spread_bias(bk),
        "bv": np.ascontiguousarray(bv.reshape(C, 1)),
    }
    in_maps = []
    for i in range(B):
        xi = np.ascontiguousarray(x[i].reshape(C, n))
        in_maps.append({"x_bf": xi.astype(_BF16), **in_common})

    trace = bool(int(os.environ.get("BASS_KERNEL_PROFILE", "0")))
    res = run_bass_kernel_spmd(nc, in_maps, core_ids=list(range(N_CORES)),
                               trace=trace)
    if trace:
        kernel.last_exec_time_ns = res.exec_time_ns
        kernel.last_results = res

    out = np.stack([res.results[i]["out"].astype(np.float32).reshape(C, hh, ww)
                    for i in range(B)])
    return out
